# revision 17
# baseline (speedup 1.0000x reference)
"""DRINetBlock Trainium2 kernel: 8-core data-parallel Bass implementation.

Strategy:
- Voxels hierarchically sorted by (scale16, scale8, scale4, scale2) segment ids so
  that every scale's segments are contiguous and contained in one 128-row tile
  (bin-packed); one order serves all 4 scales.
- 8 shards cut at scale-16 segment boundaries; 4-deep neighbor halo replicated.
- Activations live transposed [C=64, m] in SBUF; all matmuls in float32r.
- Submanifold 3x3x3 conv = dense center matmul (h @ w3[13]) + sparse corrections
  for the ~0.2% valid non-center neighbor pairs: gather rows via dma_gather from a
  DRAM mirror, per-offset-group matmuls, then scatter-add via one-hot matmuls
  (one-hot built on-chip with iota + is_equal).
- Per-scale segment mean via uploaded block-diagonal P' = I - A diag(1/n) A^T
  matmuls (bf16), giving (V - mean)^T directly.
- Attention fusion accumulated in PSUM via identity matmuls.
"""
import sys
sys.path.insert(0, '/opt/trn_rl_repo')
import numpy as np
import ml_dtypes

import concourse.bass as bass
import concourse.bacc as bacc
import concourse.mybir as mybir
import concourse.tile as tile
from concourse.bass_utils import run_bass_kernel_spmd

F32 = mybir.dt.float32
F32R = mybir.dt.float32r
BF16 = mybir.dt.bfloat16
I16 = mybir.dt.int16
I32 = mybir.dt.int32
AF = mybir.ActivationFunctionType
ALU = mybir.AluOpType

C = 64
NCLS = 20
MT = 512
NCORES = 8
LR1 = 0.01
LRV = 0.1
NITER = 4
MIRGS = 6     # mirror DMA group size (in 128-row transpose tiles)


# ---------------------------------------------------------------- host prep --

def _build_structures(nbr_idx, inv, n_cores=NCORES):
    M = nbr_idx.shape[0]
    inv = np.asarray(inv)
    nbr = np.asarray(nbr_idx)
    assert (nbr[:, 13] == np.arange(M)).all(), "center neighbor must be self"

    ks = [k for k in range(27) if k != 13]
    dst_l, src_l, k_l = [], [], []
    for ki, k in enumerate(ks):
        v = nbr[:, k] >= 0
        dst_l.append(np.nonzero(v)[0])
        src_l.append(nbr[v, k])
        k_l.append(np.full(int(v.sum()), ki, np.int32))
    dst_g = np.concatenate(dst_l).astype(np.int64)
    src_g = np.concatenate(src_l).astype(np.int64)
    k_g = np.concatenate(k_l)

    ordr = np.lexsort((np.arange(M), inv[0], inv[1], inv[2], inv[3]))
    for s in range(4):
        lab = inv[s][ordr]
        runs = 1 + np.count_nonzero(lab[1:] != lab[:-1])
        assert runs == len(np.unique(inv[s])), f"scale {s} not nested under sort"

    lab16 = inv[3][ordr]
    starts = np.concatenate([[0], np.nonzero(lab16[1:] != lab16[:-1])[0] + 1, [M]])
    run_slices = [(starts[i], starts[i + 1]) for i in range(len(starts) - 1)]
    run_lens = np.diff(starts)
    assert run_lens.max() <= 128, f"seg16 run too large: {run_lens.max()}"

    target = M / n_cores
    cuts = [0]
    acc = 0
    for i, L in enumerate(run_lens):
        acc += L
        if acc >= target * len(cuts) and len(cuts) < n_cores:
            cuts.append(i + 1)
    cuts.append(len(run_lens))

    cores = []
    for c in range(n_cores):
        runs_c = run_slices[cuts[c]:cuts[c + 1]]
        order_runs = sorted(range(len(runs_c)),
                            key=lambda i: -(runs_c[i][1] - runs_c[i][0]))
        bins, bin_free = [], []
        for ri in order_runs:
            L = runs_c[ri][1] - runs_c[ri][0]
            for bi in range(len(bins)):
                if bin_free[bi] >= L:
                    bins[bi].append(ri)
                    bin_free[bi] -= L
                    break
            else:
                bins.append([ri])
                bin_free.append(128 - L)
        own_pad = len(bins) * 128
        slots = np.full(own_pad, -1, np.int64)
        for bi, rids in enumerate(bins):
            q = bi * 128
            for ri in rids:
                a, b = runs_c[ri]
                slots[q:q + (b - a)] = ordr[a:b]
                q += b - a
        cores.append(dict(slots=slots, own_pad=own_pad))

    max_own_pad = max(cc["own_pad"] for cc in cores)
    MOWNP = -(-max_own_pad // MT) * MT

    for cc in cores:
        own_mask = np.zeros(M, bool)
        own_mask[cc["slots"][cc["slots"] >= 0]] = True
        need = own_mask.copy()
        for _ in range(4):
            sel = need[dst_g]
            nxt = need.copy()
            nxt[src_g[sel]] = True
            need = nxt
        cc["halo"] = np.nonzero(need & ~own_mask)[0]

    max_halo = max(len(cc["halo"]) for cc in cores)
    MEXT = -(-(MOWNP + max_halo) // MT) * MT
    assert MEXT < 32768, "slot ids must fit int16"
    NT_EXT = MEXT // MT
    NTO128 = MOWNP // 128

    for cc in cores:
        slot_voxel = np.full(MEXT, -1, np.int64)
        slot_voxel[:cc["own_pad"]] = cc["slots"]
        slot_voxel[MOWNP:MOWNP + len(cc["halo"])] = cc["halo"]
        g2e = np.full(M, -1, np.int64)
        vmask = slot_voxel >= 0
        g2e[slot_voxel[vmask]] = np.nonzero(vmask)[0]
        cc["slot_voxel"] = slot_voxel
        pin = (g2e[dst_g] >= 0) & (g2e[src_g] >= 0)
        cc["p_dst"] = g2e[dst_g[pin]]
        cc["p_src"] = g2e[src_g[pin]]
        cc["p_k"] = k_g[pin]

    CPK = 1
    CPT = 1
    for cc in cores:
        CPK = max(CPK, int(-(-np.bincount(cc["p_k"], minlength=26).max() // 128)))
        tcnt = np.bincount(cc["p_dst"] // MT, minlength=NT_EXT)
        CPT = max(CPT, int(-(-tcnt.max() // 128)))
    NKSLOT = 26 * CPK
    NCHUNK = NT_EXT * CPT

    for cc in cores:
        gsrc = np.zeros(NKSLOT * 128, np.int64)
        jpos = np.full(len(cc["p_dst"]), -1, np.int64)
        for ki in range(26):
            idxk = np.nonzero(cc["p_k"] == ki)[0]
            idxk = idxk[np.argsort(cc["p_dst"][idxk], kind="stable")]
            base = ki * CPK * 128
            gsrc[base:base + len(idxk)] = cc["p_src"][idxk]
            jpos[idxk] = base + np.arange(len(idxk))
        cc["gsrc"] = gsrc
        cpos = np.zeros(NCHUNK * 128, np.int64)
        dstpos = np.full((NCHUNK, 128), -1.0, np.float32)
        for T in range(NT_EXT):
            sel = np.nonzero(cc["p_dst"] // MT == T)[0]
            sel = sel[np.argsort(cc["p_dst"][sel], kind="stable")]
            for q in range(CPT):
                ch = T * CPT + q
                part = sel[q * 128:(q + 1) * 128]
                cpos[ch * 128:ch * 128 + len(part)] = jpos[part]
                dstpos[ch, :len(part)] = (cc["p_dst"][part] % MT).astype(np.float32)
        cc["cpos"] = cpos
        cc["dstpos"] = dstpos

        Pp = np.zeros((4, NTO128, 128, 128), np.float32)
        sv = cc["slot_voxel"]
        eye = np.eye(128, dtype=np.float32)
        for s in range(4):
            for t in range(NTO128):
                vox = sv[t * 128:(t + 1) * 128]
                valid = vox >= 0
                if not valid.any():
                    continue
                lab = np.where(valid, inv[s][np.clip(vox, 0, None)],
                               -1 - np.arange(128))
                eq = (lab[:, None] == lab[None, :]) & valid[:, None] & valid[None, :]
                n = eq.sum(1, keepdims=True).clip(1)
                Pp[s, t] = eye * valid[:, None] - eq / n
        cc["Pp"] = Pp.astype(np.float32)

    meta = dict(M=M, MEXT=MEXT, MOWNP=MOWNP, NT_EXT=NT_EXT,
                NTO128=NTO128, NKSLOT=NKSLOT, CPK=CPK, CPT=CPT, NCHUNK=NCHUNK)
    return cores, meta


def _fold_weights(ws):
    out = {}
    out["w1a"] = np.asarray(ws["w1a"]) * np.asarray(ws["s1a"])[:, None, :]
    out["b1a"] = np.asarray(ws["b1a"])
    out["w1b"] = np.asarray(ws["w1b"]) * np.asarray(ws["s1b"])[:, None, :]
    out["b1b"] = np.asarray(ws["b1b"])
    w3 = np.asarray(ws["w3"]) * np.asarray(ws["s3"])[:, None, None, :]
    out["w3c"] = np.ascontiguousarray(w3[:, 13])
    out["w3k"] = np.ascontiguousarray(w3[:, [k for k in range(27) if k != 13]])
    out["b3"] = np.asarray(ws["b3"])
    out["projw"] = np.asarray(ws["proj_w"]) * np.asarray(ws["proj_s"])[:, None, :]
    out["projb"] = (np.asarray(ws["proj_b"]) * np.asarray(ws["proj_s"])
                    + np.asarray(ws["proj_bb"]))
    out["attnw"] = np.asarray(ws["attn_w"]) * np.asarray(ws["attn_s"])[:, None, :]
    out["attnb"] = (np.asarray(ws["attn_b"]) * np.asarray(ws["attn_s"])
                    + np.asarray(ws["attn_bb"]))
    out["auxw"] = np.asarray(ws["aux_w"])
    out["auxb"] = np.asarray(ws["aux_b"])
    out["headw"] = np.asarray(ws["head_w"])
    out["headb"] = np.asarray(ws["head_b"])
    return {k: np.ascontiguousarray(v, np.float32) for k, v in out.items()}


def _wrap16(ix):
    """Index table layout for dma_gather/dma_scatter_add: idx j at [j%16, j//16],
    replicated to 128 partitions."""
    ix = np.asarray(ix, np.int16)
    assert len(ix) % 16 == 0
    return np.tile(ix.reshape(-1, 16).T, (8, 1)).copy()


# ------------------------------------------------------------ device program --

def _build_program(meta, relu_only=False, taps=(), taps2=False):
    MEXT, MOWNP = meta["MEXT"], meta["MOWNP"]
    NT_EXT, NTO128 = meta["NT_EXT"], meta["NTO128"]
    NKSLOT, NCHUNK, CPT = meta["NKSLOT"], meta["NCHUNK"], meta["CPT"]
    NT_OWN = MOWNP // MT
    NTE128 = MEXT // 128
    assert NTE128 % MIRGS == 0

    AFL = AF.Relu if relu_only else AF.Lrelu
    nc = bacc.Bacc("TRN2", target_bir_lowering=False)

    featsT = nc.dram_tensor("featsT", [C, MEXT], F32R, kind="ExternalInput")
    w1a = nc.dram_tensor("w1a", [NITER, C, C], F32R, kind="ExternalInput")
    w1b = nc.dram_tensor("w1b", [NITER, C, C], F32R, kind="ExternalInput")
    w3c = nc.dram_tensor("w3c", [NITER, C, C], F32R, kind="ExternalInput")
    w3k = nc.dram_tensor("w3k", [NITER, 26, C, C], F32R, kind="ExternalInput")
    bsfe = nc.dram_tensor("bsfe", [C, NITER * 3], F32, kind="ExternalInput")
    projw = nc.dram_tensor("projw", [4, C, C], F32R, kind="ExternalInput")
    projb = nc.dram_tensor("projb", [C, 4], F32, kind="ExternalInput")
    attnw = nc.dram_tensor("attnw", [4, C, C], F32R, kind="ExternalInput")
    attnb = nc.dram_tensor("attnb", [C, 4], F32, kind="ExternalInput")
    auxw = nc.dram_tensor("auxw", [C, NCLS], F32R, kind="ExternalInput")
    headw = nc.dram_tensor("headw", [C, NCLS], F32R, kind="ExternalInput")
    bhead = nc.dram_tensor("bhead", [NCLS, 2], F32, kind="ExternalInput")
    ident64r = nc.dram_tensor("ident64r", [C, C], F32R, kind="ExternalInput")
    gidx = nc.dram_tensor("gidx", [128, NKSLOT * 8], I16, kind="ExternalInput")
    cidx = nc.dram_tensor("cidx", [128, NCHUNK * 8], I16, kind="ExternalInput")
    dstpos = nc.dram_tensor("dstpos", [128, NCHUNK], F32, kind="ExternalInput")
    ppd = nc.dram_tensor("ppd", [4 * NTO128, 128, 128], F32R, kind="ExternalInput")

    mir = nc.dram_tensor("mir", [MEXT, C], F32R, kind="Internal")
    cdram = nc.dram_tensor("cdram", [NKSLOT * 128, C], F32R, kind="Internal")

    fusedT = nc.dram_tensor("fusedT", [C, MOWNP], F32, kind="ExternalOutput")
    logitsT = nc.dram_tensor("logitsT", [NCLS, MOWNP], F32, kind="ExternalOutput")
    auxT = nc.dram_tensor("auxT", [NCLS, MOWNP], F32, kind="ExternalOutput")
    tapT = {t: nc.dram_tensor(f"tap_{t}", [C, MEXT], F32, kind="ExternalOutput")
            for t in taps}
    tap2T = {}
    if taps2:
        for t in ("vm0", "os0", "sx", "ft"):
            tap2T[t] = nc.dram_tensor(f"tap_{t}", [C, MOWNP], F32, kind="ExternalOutput")
        for t in ("ms0", "a0"):
            tap2T[t] = nc.dram_tensor(f"tap_{t}", [C, MOWNP], BF16, kind="ExternalOutput")
        tap2T["vnat"] = nc.dram_tensor("tap_vnat", [128, NTO128, C], F32,
                                       kind="ExternalOutput")

    from concourse.masks import make_identity

    with tile.TileContext(nc) as tc:
        with tc.tile_pool(name="sb", bufs=1) as P1, \
             tc.tile_pool(name="sbr", bufs=2) as P2, \
             tc.tile_pool(name="sbs", bufs=2) as P3, \
             tc.tile_pool(name="sb4", bufs=1) as P4, \
             tc.tile_pool(name="pmm", bufs=2, space="PSUM") as PSmm, \
             tc.tile_pool(name="psm", bufs=2, space="PSUM") as PSsm, \
             tc.tile_pool(name="pfu", bufs=1, space="PSUM") as PSfu, \
             tc.tile_pool(name="phd", bufs=1, space="PSUM") as PShd:

            bigW = P1.tile([C, MEXT], F32R)
            W = bigW[:]

            w1a_sb = P1.tile([C, NITER * C], F32R)
            w1b_sb = P1.tile([C, NITER * C], F32R)
            w3c_sb = P1.tile([C, NITER * C], F32R)
            bsfe_sb = P1.tile([C, NITER * 3], F32)
            projw_sb = P1.tile([C, 4 * C], F32R)
            projb_sb = P1.tile([C, 4], F32)
            attnw_sb = P1.tile([C, 4 * C], F32R)
            attnb_sb = P1.tile([C, 4], F32)
            auxw_sb = P1.tile([C, NCLS], F32R)
            headw_sb = P1.tile([C, NCLS], F32R)
            bhead_sb = P1.tile([NCLS, 2], F32)
            id64r_sb = P1.tile([C, C], F32R)
            gidx_sb = P1.tile([128, NKSLOT * 8], I16)
            cidx_sb = P1.tile([128, NCHUNK * 8], I16)
            dstpos_sb = P1.tile([128, NCHUNK], F32)
            ident = P1.tile([128, 128], F32)
            make_identity(nc, ident[:])

            for dst, src in [(w1a_sb, w1a), (w1b_sb, w1b), (w3c_sb, w3c)]:
                nc.sync.dma_start(out=dst[:].rearrange("c (i d) -> c i d", i=NITER),
                                  in_=src[:].rearrange("i c d -> c i d"))
            nc.sync.dma_start(out=bsfe_sb[:], in_=bsfe[:])
            nc.sync.dma_start(out=projw_sb[:].rearrange("c (i d) -> c i d", i=4),
                              in_=projw[:].rearrange("i c d -> c i d"))
            nc.sync.dma_start(out=projb_sb[:], in_=projb[:])
            nc.sync.dma_start(out=attnw_sb[:].rearrange("c (i d) -> c i d", i=4),
                              in_=attnw[:].rearrange("i c d -> c i d"))
            nc.sync.dma_start(out=attnb_sb[:], in_=attnb[:])
            nc.sync.dma_start(out=auxw_sb[:], in_=auxw[:])
            nc.sync.dma_start(out=headw_sb[:], in_=headw[:])
            nc.sync.dma_start(out=bhead_sb[:], in_=bhead[:])
            nc.sync.dma_start(out=id64r_sb[:], in_=ident64r[:])
            nc.sync.dma_start(out=gidx_sb[:], in_=gidx[:])
            nc.sync.dma_start(out=cidx_sb[:], in_=cidx[:])
            nc.sync.dma_start(out=dstpos_sb[:], in_=dstpos[:])
            nc.sync.dma_start(out=W, in_=featsT[:])

            iota_i = P1.tile([128, MT], I32)
            nc.gpsimd.iota(iota_i[:], pattern=[[1, MT]], base=0, channel_multiplier=0)
            iota_f = P1.tile([128, MT], F32)
            nc.vector.tensor_copy(iota_f[:], iota_i[:])

            def emit_tap(name):
                if name in tapT:
                    nc.sync.dma_start(out=tapT[name][:], in_=W.bitcast(F32))

            mirv = mir[:].rearrange("(t p) c -> p t c", p=128)
            cdrv = cdram[:].rearrange("(s p) c -> p s c", p=128)

            # ------------------------------------------------ SFE iterations --
            for it in range(NITER):
                w3k_sb = P1.tile([C, 26 * C], F32R, tag="w3k")
                nc.sync.dma_start(
                    out=w3k_sb[:].rearrange("c (k d) -> c k d", k=26),
                    in_=w3k[it].rearrange("k c d -> c k d"))

                # L1: h1 = lrelu01(w1a^T @ h + b1a), in place
                for T in range(NT_EXT):
                    p = PSmm.tile([C, MT], F32, tag="mm")
                    nc.tensor.matmul(p[:], lhsT=w1a_sb[:, it * C:(it + 1) * C],
                                     rhs=W[:, T * MT:(T + 1) * MT],
                                     start=True, stop=True)
                    nc.scalar.activation(W[:, T * MT:(T + 1) * MT], p[:], AFL,
                                         bias=bsfe_sb[:, it * 3:it * 3 + 1],
                                         scale=1.0, alpha=LR1)

                emit_tap(f"l1_{it}")
                # mirror h1 -> DRAM natural
                for g in range(NTE128 // MIRGS):
                    mst = P2.tile([128, MIRGS, C], F32R, tag="mst")
                    for j in range(MIRGS):
                        t = g * MIRGS + j
                        pt = PSsm.tile([128, 128], F32, tag="sm")
                        nc.tensor.transpose(
                            out=pt[:, :C],
                            in_=W[:, t * 128:(t + 1) * 128].bitcast(F32),
                            identity=ident[:C, :C])
                        nc.vector.tensor_copy(mst[:, j, :], pt[:, :C])
                    nc.sync.dma_start(out=mirv[:, g * MIRGS:(g + 1) * MIRGS, :],
                                      in_=mst[:])

                # G gather + per-slot transposes + C matmuls
                gt = P1.tile([128, NKSLOT, C], F32R, tag="gt")
                nc.gpsimd.dma_gather(gt[:], mir[:], gidx_sb[:],
                                     NKSLOT * 128, NKSLOT * 128, C,
                                     single_packet=False)
                cst = P1.tile([128, NKSLOT, C], F32R, tag="cst")
                for s in range(NKSLOT):
                    ki = s // meta["CPK"]
                    ptg = PSsm.tile([128, 128], F32, tag="sm")
                    nc.tensor.transpose(out=ptg[:C, :],
                                        in_=gt[:, s, :].bitcast(F32),
                                        identity=ident[:, :])
                    gts = P3.tile([C, 128], F32R, tag="gts")
                    nc.vector.tensor_copy(gts[:], ptg[:C, :])
                    pc = PSsm.tile([128, 128], F32, tag="sm")
                    nc.tensor.matmul(pc[:, :C], lhsT=gts[:],
                                     rhs=w3k_sb[:, ki * C:(ki + 1) * C],
                                     start=True, stop=True)
                    nc.vector.tensor_copy(cst[:, s, :], pc[:, :C])
                nc.sync.dma_start(out=cdrv[:], in_=cst[:])

                # center + scatter, then L2 activation (in place)
                half_tiles = (NT_EXT + 1) // 2
                for half in range(2):
                    t_lo = half * half_tiles
                    t_hi = min((half + 1) * half_tiles, NT_EXT)
                    ch_lo = t_lo * CPT
                    nch = (t_hi - t_lo) * CPT
                    cs = P2.tile([128, half_tiles * CPT, C], F32R, tag="cs")
                    nc.gpsimd.dma_gather(cs[:, :nch, :], cdram[:],
                                         cidx_sb[:, ch_lo * 8:(ch_lo + nch) * 8],
                                         nch * 128, nch * 128, C,
                                         single_packet=False)
                    for T in range(t_lo, t_hi):
                        p = PSmm.tile([C, MT], F32, tag="mm")
                        nc.tensor.matmul(p[:], lhsT=w3c_sb[:, it * C:(it + 1) * C],
                                         rhs=W[:, T * MT:(T + 1) * MT],
                                         start=True, stop=False)
                        for q in range(CPT):
                            ch = T * CPT + q
                            S = P3.tile([128, MT], F32R, tag="S")
                            nc.vector.tensor_scalar(S[:], iota_f[:],
                                                    dstpos_sb[:, ch:ch + 1], None,
                                                    op0=ALU.is_equal)
                            nc.tensor.matmul(p[:], lhsT=cs[:, ch - ch_lo, :], rhs=S[:],
                                             start=False, stop=(q == CPT - 1))
                        nc.scalar.activation(W[:, T * MT:(T + 1) * MT], p[:], AFL,
                                             bias=bsfe_sb[:, it * 3 + 1:it * 3 + 2],
                                             scale=1.0, alpha=LR1)

                emit_tap(f"l2_{it}")
                # L3
                for T in range(NT_EXT):
                    p = PSmm.tile([C, MT], F32, tag="mm")
                    nc.tensor.matmul(p[:], lhsT=w1b_sb[:, it * C:(it + 1) * C],
                                     rhs=W[:, T * MT:(T + 1) * MT],
                                     start=True, stop=True)
                    nc.scalar.activation(W[:, T * MT:(T + 1) * MT], p[:], AFL,
                                         bias=bsfe_sb[:, it * 3 + 2:it * 3 + 3],
                                         scale=1.0, alpha=LR1)

            for it in range(NITER):
                pass
            emit_tap("sfe")
            # --------------------------------------------------- V + aux -----
            for T in range(NT_EXT):
                ftile = P4.tile([C, MT], F32R, tag="fre")
                nc.sync.dma_start(out=ftile[:], in_=featsT[:, T * MT:(T + 1) * MT])
                nc.vector.tensor_tensor(out=W[:, T * MT:(T + 1) * MT],
                                        in0=W[:, T * MT:(T + 1) * MT],
                                        in1=ftile[:], op=ALU.add)
                vtmp = P4.tile([C, MT], F32R, tag="vtmp")
                nc.scalar.mul(vtmp[:], W[:, T * MT:(T + 1) * MT], LRV)
                nc.vector.tensor_tensor(out=W[:, T * MT:(T + 1) * MT],
                                        in0=W[:, T * MT:(T + 1) * MT],
                                        in1=vtmp[:], op=ALU.max)

            emit_tap("V")
            # V natural (bf16) for P' matmuls
            vnat = P1.tile([128, NTO128, C], F32R, tag="vnat")
            for t in range(NTO128):
                pt = PSsm.tile([128, 128], F32, tag="vtr")
                nc.tensor.transpose(out=pt[:, :C],
                                    in_=W[:, t * 128:(t + 1) * 128].bitcast(F32),
                                    identity=ident[:C, :C])
                nc.scalar.copy(vnat[:, t, :], pt[:, :C])
            for T in range(NT_OWN):
                pa = PShd.tile([NCLS, MT], F32, tag="hd")
                nc.tensor.matmul(pa[:], lhsT=auxw_sb[:],
                                 rhs=W[:, T * MT:(T + 1) * MT],
                                 start=True, stop=True)
                at = P4.tile([NCLS, MT], F32, tag="lt")
                nc.scalar.activation(at[:], pa[:], AF.Identity,
                                     bias=bhead_sb[:, 0:1], scale=1.0)
                nc.sync.dma_start(out=auxT[:, T * MT:(T + 1) * MT], in_=at[:])

            if taps2:
                nc.sync.dma_start(out=tap2T["vnat"][:], in_=vnat[:].bitcast(F32))
            # ------------------------------------- scales + fusion + heads ---
            for T in range(NT_OWN):
                Wt = W[:, T * MT:(T + 1) * MT]
                ms = []
                for s in range(4):
                    pm = PSmm.tile([C, MT], F32, tag="mm")
                    for tt in range(MT // 128):
                        t = T * (MT // 128) + tt
                        ppt = P3.tile([128, 128], F32R, tag="ppt")
                        nc.sync.dma_start(out=ppt[:], in_=ppd[s * NTO128 + t])
                        nc.tensor.matmul(pm[:, tt * 128:(tt + 1) * 128],
                                         lhsT=vnat[:, t, :], rhs=ppt[:],
                                         start=True, stop=True)
                    if taps2 and s == 0:
                        vmt = P3.tile([C, MT], F32, tag="vmt")
                        nc.vector.tensor_copy(vmt[:], pm[:])
                        nc.sync.dma_start(out=tap2T["vm0"][:, T*MT:(T+1)*MT], in_=vmt[:])
                    ost = P3.tile([C, MT], F32R, tag="ost")
                    nc.vector.tensor_tensor(out=ost[:], in0=pm[:], in1=Wt,
                                            op=ALU.mult)
                    if taps2 and s == 0:
                        nc.sync.dma_start(out=tap2T["os0"][:, T*MT:(T+1)*MT], in_=ost[:].bitcast(F32))
                    pj = PSmm.tile([C, MT], F32, tag="mm")
                    nc.tensor.matmul(pj[:], lhsT=projw_sb[:, s * C:(s + 1) * C],
                                     rhs=ost[:], start=True, stop=True)
                    mst_ = P3.tile([C, MT], BF16, tag=f"ms{s}")
                    nc.scalar.activation(mst_[:], pj[:], AFL,
                                         bias=projb_sb[:, s:s + 1],
                                         scale=1.0, alpha=LR1)
                    if taps2 and s == 0:
                        nc.sync.dma_start(out=tap2T["ms0"][:, T*MT:(T+1)*MT], in_=mst_[:])
                    ms.append(mst_)
                sx01 = P3.tile([C, MT], BF16, tag="sx01")
                nc.vector.tensor_tensor(out=sx01[:], in0=ms[0][:], in1=ms[1][:],
                                        op=ALU.add)
                sx23 = P3.tile([C, MT], BF16, tag="sx23")
                nc.vector.tensor_tensor(out=sx23[:], in0=ms[2][:], in1=ms[3][:],
                                        op=ALU.add)
                sx = P3.tile([C, MT], F32R, tag="sx")
                nc.vector.tensor_tensor(out=sx[:], in0=sx01[:], in1=sx23[:],
                                        op=ALU.add)
                if taps2:
                    nc.sync.dma_start(out=tap2T["sx"][:, T*MT:(T+1)*MT], in_=sx[:].bitcast(F32))
                pf = PSfu.tile([C, MT], F32, tag="fu")
                for s in range(4):
                    pa = PSmm.tile([C, MT], F32, tag="mm")
                    nc.tensor.matmul(pa[:], lhsT=attnw_sb[:, s * C:(s + 1) * C],
                                     rhs=sx[:], start=True, stop=True)
                    a_t = P3.tile([C, MT], BF16, tag="a")
                    nc.scalar.activation(a_t[:], pa[:], AF.Sigmoid,
                                         bias=attnb_sb[:, s:s + 1], scale=1.0)
                    if taps2 and s == 0:
                        nc.sync.dma_start(out=tap2T["a0"][:, T*MT:(T+1)*MT], in_=a_t[:])
                    t_t = P3.tile([C, MT], F32R, tag="t")
                    nc.vector.tensor_tensor(out=t_t[:], in0=a_t[:], in1=ms[s][:],
                                            op=ALU.mult)
                    nc.tensor.matmul(pf[:], lhsT=id64r_sb[:], rhs=t_t[:],
                                     start=(s == 0), stop=(s == 3))
                ft = P3.tile([C, MT], F32R, tag="ft")
                nc.scalar.copy(ft[:], pf[:])
                if taps2:
                    nc.sync.dma_start(out=tap2T["ft"][:, T*MT:(T+1)*MT], in_=ft[:].bitcast(F32))
                nc.sync.dma_start(out=fusedT[:, T * MT:(T + 1) * MT],
                                  in_=ft[:].bitcast(F32))
                ph = PShd.tile([NCLS, MT], F32, tag="hd")
                nc.tensor.matmul(ph[:], lhsT=headw_sb[:], rhs=ft[:],
                                 start=True, stop=True)
                lt = P4.tile([NCLS, MT], F32, tag="lt")
                nc.scalar.activation(lt[:], ph[:], AF.Identity,
                                     bias=bhead_sb[:, 1:2], scale=1.0)
                nc.sync.dma_start(out=logitsT[:, T * MT:(T + 1) * MT], in_=lt[:])

    nc.finalize()
    return nc


# ------------------------------------------------------------------- kernel --

def _make_in_maps(cores, meta, W, feats):
    MEXT = meta["MEXT"]
    shared = {
        "w1a": W["w1a"], "w1b": W["w1b"], "w3c": W["w3c"], "w3k": W["w3k"],
        "bsfe": np.concatenate([np.stack([W["b1a"][i], W["b3"][i], W["b1b"][i]], 1)
                                for i in range(NITER)], axis=1).astype(np.float32),
        "projw": W["projw"], "projb": W["projb"].T.copy(),
        "attnw": W["attnw"], "attnb": W["attnb"].T.copy(),
        "auxw": W["auxw"], "headw": W["headw"],
        "bhead": np.stack([W["auxb"], W["headb"]], 1).astype(np.float32),
        "ident64r": np.eye(C, dtype=np.float32),
    }
    in_maps = []
    for cc in cores:
        sv = cc["slot_voxel"]
        fT = np.zeros((C, MEXT), np.float32)
        m = sv >= 0
        fT[:, m] = feats[sv[m]].T
        im = dict(shared)
        im["featsT"] = fT
        im["gidx"] = _wrap16(cc["gsrc"])
        im["cidx"] = _wrap16(cc["cpos"])
        im["dstpos"] = cc["dstpos"].T.copy()
        im["ppd"] = cc["Pp"].reshape(4 * meta["NTO128"], 128, 128)
        in_maps.append(im)
    return in_maps


_prog_cache = {}


def _get_program(meta):
    key = tuple(sorted(meta.items()))
    if key not in _prog_cache:
        _prog_cache[key] = _build_program(meta)
    return _prog_cache[key]


def kernel(**inputs):
    inputs = {k: np.asarray(v) for k, v in inputs.items()}
    feats = inputs["feats"].astype(np.float32)
    M = feats.shape[0]
    cores, meta = _build_structures(inputs["nbr_idx"], inputs["inv"])
    W = _fold_weights(inputs)
    in_maps = _make_in_maps(cores, meta, W, feats)
    nc = _get_program(meta)
    res = run_bass_kernel_spmd(nc, in_maps, core_ids=list(range(NCORES)))

    fused = np.zeros((M, C), np.float32)
    logits = np.zeros((M, NCLS), np.float32)
    aux = np.zeros((M, NCLS), np.float32)
    for cc, r in zip(cores, res.results):
        sl = cc["slot_voxel"][:meta["MOWNP"]]
        m = sl >= 0
        fused[sl[m]] = r["fusedT"][:, m].T
        logits[sl[m]] = r["logitsT"][:, m].T
        aux[sl[m]] = r["auxT"][:, m].T
    return fused, logits, aux
